# revision 7
# baseline (speedup 1.0000x reference)
"""Trainium2 Bass kernel for Exercise-KC GraphConvolution (concat=True branch).

Computes: elu((adj @ (kc_h @ W1)) * (ex_h @ W1 @ W2))   -> [50000, 512]

Strategy (8 NeuronCores):
  - Shard exercise rows across cores: pad 50000 -> 50176 = 8 * 49 * 128.
  - Batch-independent weight folding on host: kcWh = kc_h @ W1 ([2048, 512])
    and W12 = W1 @ W2 ([512, 512]) are precomputed in fp32 and shipped to the
    device, removing all setup matmuls from the kernel.
  - Everything streams in fp16 (measured end-to-end rel err ~4.6e-4 vs the
    2e-2 gate): half the HBM traffic of fp32/fp32r at the same PE rate
    (1 col/cycle), and FWL (fast weight load) fully hides LDWEIGHTS.
  - The startup transient is DMA-bound (~4MB of weights + first tiles must
    land before the spmm can run at speed).  To keep the PE fed, the
    exercise stream ([P, 4, 128] fp16 chunks, 128KB/tile) is separate from
    the adjacency stream (512KB/tile): the exercise branches of the first
    EX_PREFILL tiles run during the startup window (they need only W12 and
    tiny ex chunks), parking ex_h @ W12 results in SBUF as fp16, and the
    spmm phase starts once kcWh has landed, gap-free.
  - Per 128-row output tile: 4 accumulating matmuls for the exercise branch
    (K=512) then 16 for the spmm (K=2048), all N=512, then
    elu(x) = max(x, exp(min(x,0)) - 1) on vector+scalar engines.
  - Last tile is split into two N=256 halves so the final elementwise chain
    and store overlap the last spmm matmuls (shorter kernel tail).
"""

import numpy as np

import concourse.bass as bass
import concourse.mybir as mybir
import concourse.tile as tile
from concourse import bacc
from concourse.bass_utils import run_bass_kernel_spmd

N_EX = 50000
IN_F = 512
OUT_F = 512
N_KC = 2048
N_CORES = 8

P = 128                       # partitions
T = 49                        # row-tiles per core
E_PER_CORE = T * P            # 6272
E_PAD = N_CORES * E_PER_CORE  # 50176
KHI_ADJ = N_KC // P           # 16
KHI_IN = IN_F // P            # 4
FB = OUT_F                    # 512 (psum free dim)

W_PRE = 12                    # tiles whose ex-branch runs during startup

F32 = mybir.dt.float32
F16 = mybir.dt.float16


def build_nc(n_tiles: int = T):
    """Build + compile the per-core Bass program (same program on all cores)."""
    nc = bacc.Bacc(
        "TRN2",
        target_bir_lowering=False,
        debug=False,
        enable_asserts=False,
        num_devices=N_CORES,
    )
    AF = mybir.ActivationFunctionType
    OP = mybir.AluOpType

    adjs = nc.dram_tensor("adjs", [n_tiles, P, KHI_ADJ, P], F16,
                          kind="ExternalInput")
    exs = nc.dram_tensor("exs", [n_tiles, P, KHI_IN, P], F16,
                         kind="ExternalInput")
    # kcWh = kc_h @ W1 (host-folded), k-major chunks [ki][k_lo][n]
    kcw = nc.dram_tensor("kcw", [KHI_ADJ, P, FB], F16, kind="ExternalInput")
    # W12 = W1 @ W2 (host-folded), k-major [k_lo][kj][n]
    w12 = nc.dram_tensor("w12", [P, KHI_IN, FB], F16, kind="ExternalInput")
    outp = nc.dram_tensor("outp", [n_tiles, P, FB], F16, kind="ExternalOutput")

    def ring(i):
        return nc.sync if i % 2 == 0 else nc.scalar

    with tile.TileContext(nc) as tc:
        with (
            tc.tile_pool(name="const", bufs=1) as constp,
            tc.tile_pool(name="adj", bufs=6) as adjp,
            tc.tile_pool(name="exg", bufs=W_PRE + 4) as exgp,
            tc.tile_pool(name="exb", bufs=W_PRE + 4) as exbp,
            tc.tile_pool(name="outb", bufs=3) as outbp,
            tc.tile_pool(name="tmp", bufs=3) as tmpp,
            tc.tile_pool(name="ps", bufs=2, space=bass.MemorySpace.PSUM) as psp,
            tc.tile_pool(name="psq", bufs=2, space=bass.MemorySpace.PSUM) as psq,
        ):
            # PE warm-up: the HAM clock gate needs ~3.4us of activity to lift
            # the 1.2GHz cold throttle; burn it on a zero tile while the
            # first DMAs are still in flight.
            warm_sb = constp.tile([P, P], F16)
            nc.vector.memset(warm_sb[:], 0.0)
            for _ in range(24):
                pwu = psq.tile([P, P], F32, tag="pw")
                nc.tensor.matmul(pwu[:], warm_sb[:], warm_sb[:],
                                 start=True, stop=True)

            # --- startup DMA schedule ---
            # w12 + the first ex chunks lead (tiny, unblock the ex branches),
            # kcWh chunks next split across both rings, adjacency follows.
            w12_sb = constp.tile([P, KHI_IN, FB], F16)
            nc.scalar.dma_start(w12_sb[:], w12[:])
            ex_sbs = {}
            for t in range(W_PRE + 1):
                eg = exgp.tile([P, KHI_IN, P], F16)
                ring(t).dma_start(eg[:], exs[t])
                ex_sbs[t] = eg
            kcw_sbs = []
            for ki in range(KHI_ADJ):
                kg = constp.tile([P, FB], F16, tag=f"kcw{ki}")
                ring(ki + 1).dma_start(kg[:], kcw[ki])
                kcw_sbs.append(kg)
            adj_sbs = {}
            for t in range(W_PRE + 1):
                ag = adjp.tile([P, KHI_ADJ, P], F16)
                ring(t).dma_start(ag[:], adjs[t])
                adj_sbs[t] = ag

            exb_sbs = {}

            def ex_branch(t):
                """ex_h @ W12 for tile t -> fp16 SBUF tile (via PSUM copy)."""
                eg = ex_sbs.pop(t)
                ps_e = psp.tile([P, FB], F32, tag="pse")
                for kj in range(KHI_IN):
                    nc.tensor.matmul(
                        ps_e[:],
                        eg[:, kj, :],
                        w12_sb[:, kj, :],
                        start=(kj == 0),
                        stop=(kj == KHI_IN - 1),
                    )
                exb = exbp.tile([P, FB], F16)
                nc.scalar.copy(exb[:], ps_e[:])  # one PSUM operand max
                exb_sbs[t] = exb

            def elementwise(t, ps_s, exb_ap, o_sb, sl):
                # elu(prod) = max(prod, exp(min(prod,0)) - 1)
                n = sl.stop - sl.start
                prod = tmpp.tile([P, n], F32, tag=f"prod{n}")
                nc.vector.tensor_tensor(prod[:], ps_s[:], exb_ap, OP.mult)
                nmin = tmpp.tile([P, n], F32, tag=f"nmin{n}")
                nc.vector.tensor_scalar(nmin[:], prod[:], 0.0, None, OP.min)
                expv = tmpp.tile([P, n], F32, tag=f"expv{n}")
                nc.scalar.activation(expv[:], nmin[:], AF.Exp)
                nc.vector.scalar_tensor_tensor(
                    o_sb, expv[:], -1.0, prod[:], OP.add, OP.max)

            # ex branches of the first tiles run while kcWh is in flight
            for t in range(W_PRE + 1):
                ex_branch(t)

            # ---- main loop over row-tiles ----
            for t in range(n_tiles):
                if t > W_PRE:
                    eg = exgp.tile([P, KHI_IN, P], F16)
                    ring(t).dma_start(eg[:], exs[t])
                    ex_sbs[t] = eg
                    ag = adjp.tile([P, KHI_ADJ, P], F16)
                    ring(t + 1).dma_start(ag[:], adjs[t])
                    adj_sbs[t] = ag
                    ex_branch(t)

                a_sb = adj_sbs.pop(t)
                exb = exb_sbs.pop(t)
                if t < n_tiles - 1:
                    ps_s = psp.tile([P, FB], F32, tag="pss")  # spmm branch
                    for ki in range(KHI_ADJ):
                        nc.tensor.matmul(
                            ps_s[:],
                            a_sb[:, ki, :],
                            kcw_sbs[ki][:],
                            start=(ki == 0),
                            stop=(ki == KHI_ADJ - 1),
                        )
                    o_sb = outbp.tile([P, FB], F16)
                    elementwise(t, ps_s, exb[:], o_sb[:], slice(0, FB))
                    ring(t).dma_start(outp[t], o_sb[:])
                else:
                    # last tile: split N in half so the elementwise chain and
                    # store of half 0 overlap the spmm matmuls of half 1
                    HF = FB // 2
                    for h in range(2):
                        sl = slice(h * HF, (h + 1) * HF)
                        ps_h = psp.tile([P, HF], F32, tag="pslast")
                        for ki in range(KHI_ADJ):
                            nc.tensor.matmul(
                                ps_h[:],
                                a_sb[:, ki, :],
                                kcw_sbs[ki][:, sl],
                                start=(ki == 0),
                                stop=(ki == KHI_ADJ - 1),
                            )
                        o_sb = outbp.tile([P, HF], F16, tag="olast")
                        elementwise(t, ps_h, exb[:, sl], o_sb[:], sl)
                        ring(h).dma_start(outp[t, :, sl], o_sb[:])

    nc.compile()
    return nc


def prep_inputs(exercise_h, kc_h, adj_exercise_kc, W1, W2,
                n_tiles: int = T):
    """Host-side shard + layout prep. Returns in_maps (one dict per core)."""
    ex = np.asarray(exercise_h, dtype=np.float32)
    kc = np.asarray(kc_h, dtype=np.float32)
    adj = np.asarray(adj_exercise_kc, dtype=np.float32)
    w1 = np.asarray(W1, dtype=np.float32)
    w2 = np.asarray(W2, dtype=np.float32)

    # batch-independent weight folding (exact fp32, then one fp16 rounding)
    kcwh = (kc @ w1).astype(np.float16)                    # [2048, 512]
    w12 = (w1 @ w2).astype(np.float16)                     # [512, 512]

    e_pad = N_CORES * n_tiles * P
    n_rows = min(N_EX, e_pad)

    adj_p = np.zeros((e_pad, N_KC), np.float16)
    adj_p[:n_rows] = adj[:n_rows]
    ex_p = np.zeros((e_pad, IN_F), np.float16)
    ex_p[:n_rows] = ex[:n_rows]
    # [core, t, k_lo, k_hi, m] (k-major chunks, stationary-operand layout)
    adjs = np.ascontiguousarray(
        adj_p.reshape(N_CORES, n_tiles, P, KHI_ADJ, P).transpose(0, 1, 4, 3, 2))
    exs = np.ascontiguousarray(
        ex_p.reshape(N_CORES, n_tiles, P, KHI_IN, P).transpose(0, 1, 4, 3, 2))

    kcw = np.ascontiguousarray(kcwh.reshape(KHI_ADJ, P, FB))
    w12r = np.ascontiguousarray(w12.reshape(KHI_IN, P, FB).transpose(1, 0, 2))

    return [
        {"adjs": adjs[c], "exs": exs[c], "kcw": kcw, "w12": w12r}
        for c in range(N_CORES)
    ]


def unpack_output(results, n_tiles: int = T) -> np.ndarray:
    """results: list per core of {"outp": [n_tiles, P, FB]} -> [N_EX, FB]."""
    per_core = [
        np.asarray(r["outp"]).reshape(n_tiles * P, FB)
        for r in results
    ]
    return np.concatenate(per_core, axis=0)[:N_EX].astype(np.float32)


_NC_CACHE: dict = {}


def _get_nc():
    if T not in _NC_CACHE:
        _NC_CACHE[T] = build_nc()
    return _NC_CACHE[T]


def kernel(exercise_h, kc_h, adj_exercise_kc, W1, W2):
    nc = _get_nc()
    in_maps = prep_inputs(exercise_h, kc_h, adj_exercise_kc, W1, W2)
    res = run_bass_kernel_spmd(nc, in_maps, core_ids=list(range(N_CORES)))
    return np.ascontiguousarray(unpack_output(res.results))
